# revision 25
# baseline (speedup 1.0000x reference)
"""Trainium2 Bass kernel for BinaryHead: logits = (l2norm(fea) @ W.T + b) * 16.

Sharding: data-parallel over the batch dim across 8 NeuronCores (2048 rows
each).  The host stages each core's shard TRANSPOSED ([emb, batch], a layout
choice) so the embedding/contraction dim lands on SBUF partitions, which is
what the TensorEngine contracts over.  Per core the device kernel streams 16
row-panels [128e x 2048b]:

  z.T[c, b]   += Wt_chunk.T @ panel          (4-col stationary, panel moving)
  sumsq[b]    += ones.T @ panel**2           (ACT squares the panel first)

and a small epilogue computes out.T = z.T * (S/sqrt(sumsq)) + S*b on device.
The normalization never touches the big tensor: out = z*S/norm + S*b.

Matmuls run as float32r (full fp32 data, single-pass PE mode): the moving
operand streams at 1 cycle/row vs 4 for classic fp32, keeping PE (~28us)
under the HBM roofline (~47us/core for the 16MiB shard).
"""

import os
from contextlib import ExitStack

import numpy as np

NUM_CLASS = 4
EMB = 2048
BATCH = 16384
N_CORES = 8
ROWS = BATCH // N_CORES  # 2048 rows per core
S = 16.0

N_ETILES = EMB // 128  # 16 e-panels per core
N_BCHUNK = ROWS // 512  # 4 psum-width chunks of the batch

# compute dtype config: "fp32" (f32 data, f32r matmuls) or "bf16"
DTYPE_CFG = os.environ.get("KERNEL_DTYPE", "fp32")

_CACHE = {}


def _build_nc():
    import concourse.bacc as bacc
    import concourse.bass as bass
    import concourse.mybir as mybir
    import concourse.tile as tile
    from concourse.hw_specs import get_activation_tables

    f32 = mybir.dt.float32
    f32r = mybir.dt.float32r
    bf16 = mybir.dt.bfloat16
    use_bf16 = DTYPE_CFG == "bf16"
    # fp32 config: full-precision data streamed as float32r (single-pass PE
    # mode, 1 cyc/row); squares for sumsq go through bf16 (plenty of
    # precision for a sum of positives).
    dt_data = bf16 if use_bf16 else f32r
    dt_sq = bf16

    nc = bacc.Bacc(
        "TRN2",
        target_bir_lowering=False,
        debug=False,
        enable_asserts=False,
        num_devices=N_CORES,
    )

    feaT = nc.dram_tensor("feaT", [EMB, ROWS], dt_data, kind="ExternalInput").ap()
    wt = nc.dram_tensor(
        "wt", [128, N_ETILES * NUM_CLASS], dt_data, kind="ExternalInput"
    ).ap()
    onesv = nc.dram_tensor("onesv", [128, 1], dt_sq, kind="ExternalInput").ap()
    sones = nc.dram_tensor("sones", [1, NUM_CLASS], f32, kind="ExternalInput").ap()
    sbias = nc.dram_tensor("sbias", [NUM_CLASS, 1], f32, kind="ExternalInput").ap()
    outT = nc.dram_tensor("outT", [NUM_CLASS, ROWS], f32, kind="ExternalOutput").ap()



    with tile.TileContext(nc) as tc, ExitStack() as ctx:
        pconst = ctx.enter_context(tc.tile_pool(name="pconst", bufs=1))
        pdata = ctx.enter_context(tc.tile_pool(name="pdata", bufs=5))
        psq = ctx.enter_context(tc.tile_pool(name="psq", bufs=6))
        pep = ctx.enter_context(tc.tile_pool(name="pep", bufs=1))
        pz = ctx.enter_context(tc.tile_pool(name="pz", bufs=1, space="PSUM"))
        ps = ctx.enter_context(tc.tile_pool(name="ps", bufs=1, space="PSUM"))

        # consts go through the SWDGE ring so the sync HWDGE ring starts the
        # big data streams immediately
        wt_s = pconst.tile([128, N_ETILES * NUM_CLASS], dt_data)
        nc.gpsimd.dma_start(out=wt_s, in_=wt)
        ones_s = pconst.tile([128, 1], dt_sq)
        nc.gpsimd.dma_start(out=ones_s, in_=onesv)
        sones_s = pconst.tile([1, NUM_CLASS], f32)
        nc.gpsimd.dma_start(out=sones_s, in_=sones)
        sbias_s = pconst.tile([NUM_CLASS, 1], f32)
        nc.gpsimd.dma_start(out=sbias_s, in_=sbias)
        zero1_s = pconst.tile([1, 1], f32)
        nc.vector.memset(zero1_s, 0.0)
        zero128_s = pconst.tile([128, 1], f32)
        nc.vector.memset(zero128_s, 0.0)
        # rsqrt via exp(-0.5*ln(ss) + ln(S)): folds the *S scale in for free
        lnS_s = pconst.tile([1, 1], f32)
        nc.vector.memset(lnS_s, float(np.log(S)))

        # preload the one ACT table set that covers Square+Ln+Exp so no
        # table switch lands on the critical path (greedy per-function
        # selection would otherwise load 3 sets)
        nlx_id = list(get_activation_tables(nc.m.arch)).index(
            "natural_log_exp_and_others"
        )
        nc.scalar.add_instruction(
            mybir.InstLoadActFuncSet(
                name=f"I-{nc.next_id()}", act_func_set_id=nlx_id
            )
        )

        # accumulators: z.T as one 4-bank tensor (PE-only writers), sumsq as
        # four single-bank tensors so the epilogue rnb reuse pipelines
        zt_ps = pz.tile([NUM_CLASS, ROWS], f32, tag="zt")
        ss_ps = [
            ps.tile([1, 512], f32, tag="ssrnb", bufs=4, name=f"ss{j}")
            for j in range(N_BCHUNK)
        ]

        # panel groups: single-panel first DMAs so the PE starts early, then
        # 2-panel (2MB fp32) transfers for bandwidth
        groups = [(0,), (1,)] + [
            (t, t + 1) for t in range(2, N_ETILES, 2)
        ]
        for g in groups:
            npan = len(g)
            xt = pdata.tile([128, 2, ROWS], dt_data, tag="xt")
            nc.sync.dma_start(
                out=xt[:, :npan, :],
                in_=feaT[g[0] * 128 : (g[-1] + 1) * 128, :].rearrange(
                    "(a p) b -> p a b", p=128
                ),
            )
            x2 = psq.tile([128, 2, ROWS], dt_sq, tag="x2")
            for a in range(npan):
                # alternate square engine per panel: ACT and DVE run them
                # concurrently
                xin = xt[:, a, :] if use_bf16 else xt[:, a, :].bitcast(f32)
                if g[a] % 2 == 0:
                    nc.scalar.activation(
                        out=x2[:, a, :],
                        in_=xin,
                        func=mybir.ActivationFunctionType.Square,
                        bias=zero128_s,
                        scale=1.0,
                    )
                else:
                    nc.vector.tensor_mul(x2[:, a, :], xin, xin)

            # z matmuls first (need only xt), ss matmuls after (need x2)
            for a in range(npan):
                t = g[a]
                start = t == 0
                stop = t == N_ETILES - 1
                wchunk = wt_s[:, t * NUM_CLASS : (t + 1) * NUM_CLASS]
                for j in range(N_BCHUNK):
                    bsl = slice(j * 512, (j + 1) * 512)
                    nc.tensor.matmul(
                        zt_ps[:, bsl], wchunk, xt[:, a, bsl], start=start, stop=stop
                    )
            for a in range(npan):
                t = g[a]
                start = t == 0
                stop = t == N_ETILES - 1
                for j in range(N_BCHUNK):
                    bsl = slice(j * 512, (j + 1) * 512)
                    nc.tensor.matmul(
                        ss_ps[j], ones_s, x2[:, a, bsl], start=start, stop=stop
                    )

        # epilogue: out.T[c,b] = z.T[c,b] * S/sqrt(sumsq[b]) + S*bias[c]
        # S/sqrt(ss) = exp(-0.5*ln(ss) + ln(S)): two 1-cyc/elem ACT ops from
        # one table set, avoiding the slow iterative DVE reciprocal.  The
        # multiply and bias-add run on DVE so ACT only ever loads one set.
        lnss_s = pep.tile([1, ROWS], f32)
        rnorm_s = pep.tile([1, ROWS], f32)
        z_s = pep.tile([NUM_CLASS, ROWS], f32)
        zr_s = pep.tile([NUM_CLASS, ROWS], f32)
        out_s = pep.tile([NUM_CLASS, ROWS], f32)
        # copy z.T out of psum as soon as its accumulation stops
        nc.vector.tensor_copy(z_s, zt_ps)
        for j in range(N_BCHUNK):
            bsl = slice(j * 512, (j + 1) * 512)
            nc.scalar.activation(
                out=lnss_s[:, bsl],
                in_=ss_ps[j],
                func=mybir.ActivationFunctionType.Ln,
                bias=zero1_s,
                scale=1.0,
            )
        for j in range(N_BCHUNK):
            bsl = slice(j * 512, (j + 1) * 512)
            nc.scalar.activation(
                out=rnorm_s[:, bsl],
                in_=lnss_s[:, bsl],
                func=mybir.ActivationFunctionType.Exp,
                bias=lnS_s,
                scale=-0.5,
            )
        # broadcast S/norm across the 4 class partitions via k=1 matmuls
        # (each reuses a freed sumsq psum bank; separate tensors so the PE
        # writes pipeline with the DVE reads)
        rnb = [
            ps.tile([NUM_CLASS, 512], f32, tag="ssrnb", bufs=4, name=f"rnb{j}")
            for j in range(N_BCHUNK)
        ]
        for j in range(N_BCHUNK):
            bsl = slice(j * 512, (j + 1) * 512)
            nc.tensor.matmul(
                rnb[j], sones_s, rnorm_s[:, bsl], start=True, stop=True
            )
            nc.vector.tensor_mul(zr_s[:, bsl], z_s[:, bsl], rnb[j])
            nc.vector.tensor_scalar_add(
                out_s[:, bsl], in0=zr_s[:, bsl], scalar1=sbias_s
            )
        nc.sync.dma_start(out=outT, in_=out_s)

    nc.compile()
    return nc


def _get_nc():
    if "nc" not in _CACHE:
        _CACHE["nc"] = _build_nc()
    return _CACHE["nc"]


def _stage_inputs(fea, W, b):
    import ml_dtypes

    np_data = ml_dtypes.bfloat16 if DTYPE_CFG == "bf16" else np.float32
    fea = np.asarray(fea, dtype=np.float32)
    W = np.asarray(W, dtype=np.float32)
    b = np.asarray(b, dtype=np.float32)

    # wt[p, 4t+c] = W[c, 128t+p]
    wt = np.ascontiguousarray(
        W.reshape(NUM_CLASS, N_ETILES, 128).transpose(2, 1, 0).reshape(128, -1)
    ).astype(np_data)
    onesv = np.ones((128, 1), dtype=ml_dtypes.bfloat16)
    # the *S scale is folded into the exp(-0.5*ln(ss)+ln(S)) rsqrt, so the
    # class-broadcast matmul uses plain ones
    sones = np.ones((1, NUM_CLASS), dtype=np.float32)
    sbias = (S * b).reshape(NUM_CLASS, 1).astype(np.float32)

    in_maps = []
    for i in range(N_CORES):
        shard = fea[i * ROWS : (i + 1) * ROWS, :]
        feaT = np.ascontiguousarray(shard.T).astype(np_data)
        in_maps.append(
            {"feaT": feaT, "wt": wt, "onesv": onesv, "sones": sones, "sbias": sbias}
        )
    return in_maps


def run(fea, W, b, trace=False):
    from concourse.bass_utils import run_bass_kernel_spmd

    nc = _get_nc()
    in_maps = _stage_inputs(fea, W, b)
    res = run_bass_kernel_spmd(
        nc, in_maps, core_ids=list(range(N_CORES)), trace=trace
    )
    out = np.empty((BATCH, NUM_CLASS), dtype=np.float32)
    for i in range(N_CORES):
        out[i * ROWS : (i + 1) * ROWS, :] = res.results[i]["outT"].T
    return out, res


def kernel(fea, W, b):
    out, _ = run(fea, W, b, trace=False)
    return out


# revision 29
# speedup vs baseline: 1.1031x; 1.1031x over previous
"""Trainium2 Bass kernel for BinaryHead: logits = (l2norm(fea) @ W.T + b) * 16.

Sharding: data-parallel over the batch dim across 8 NeuronCores (2048 rows
each).  The host stages each core's shard TRANSPOSED ([emb, batch], a layout
choice) so the embedding/contraction dim lands on SBUF partitions, which is
what the TensorEngine contracts over.  Per core the device kernel streams 16
row-panels [128e x 2048b]:

  z.T[c, b]   += Wt_chunk.T @ panel          (4-col stationary, panel moving)
  sumsq[b]    += ones.T @ panel**2           (ACT squares the panel first)

and a small epilogue computes out.T = z.T * (S/sqrt(sumsq)) + S*b on device.
The normalization never touches the big tensor: out = z*S/norm + S*b.

Matmuls run as float32r (full fp32 data, single-pass PE mode): the moving
operand streams at 1 cycle/row vs 4 for classic fp32, keeping PE (~28us)
under the HBM roofline (~47us/core for the 16MiB shard).
"""

import os
from contextlib import ExitStack

import numpy as np

NUM_CLASS = 4
EMB = 2048
BATCH = 16384
N_CORES = 8
ROWS = BATCH // N_CORES  # 2048 rows per core
S = 16.0

N_ETILES = EMB // 128  # 16 e-panels per core
N_BCHUNK = ROWS // 512  # 4 psum-width chunks of the batch

# compute dtype config: "fp32" (f32 data, f32r matmuls) or "bf16"
DTYPE_CFG = os.environ.get("KERNEL_DTYPE", "fp32")

_CACHE = {}


def _build_nc():
    import concourse.bacc as bacc
    import concourse.bass as bass
    import concourse.mybir as mybir
    import concourse.tile as tile
    from concourse.hw_specs import get_activation_tables

    f32 = mybir.dt.float32
    f32r = mybir.dt.float32r
    bf16 = mybir.dt.bfloat16
    use_bf16 = DTYPE_CFG == "bf16"
    # fp32 config: full-precision data streamed as float32r (single-pass PE
    # mode, 1 cyc/row); squares for sumsq go through bf16 (plenty of
    # precision for a sum of positives).
    dt_data = bf16 if use_bf16 else f32r
    dt_sq = bf16

    nc = bacc.Bacc(
        "TRN2",
        target_bir_lowering=False,
        debug=False,
        enable_asserts=False,
        num_devices=N_CORES,
    )

    feaT = nc.dram_tensor("feaT", [EMB, ROWS], dt_data, kind="ExternalInput").ap()
    wt = nc.dram_tensor(
        "wt", [128, N_ETILES * NUM_CLASS], dt_data, kind="ExternalInput"
    ).ap()
    onesv = nc.dram_tensor("onesv", [128, 1], dt_sq, kind="ExternalInput").ap()
    sones = nc.dram_tensor("sones", [1, NUM_CLASS], f32r, kind="ExternalInput").ap()
    sbias = nc.dram_tensor("sbias", [NUM_CLASS, 1], f32, kind="ExternalInput").ap()
    outT = nc.dram_tensor("outT", [NUM_CLASS, ROWS], f32, kind="ExternalOutput").ap()



    with tile.TileContext(nc) as tc, ExitStack() as ctx:
        pconst = ctx.enter_context(tc.tile_pool(name="pconst", bufs=1))
        pdata = ctx.enter_context(tc.tile_pool(name="pdata", bufs=5))
        psq = ctx.enter_context(tc.tile_pool(name="psq", bufs=6))
        pep = ctx.enter_context(tc.tile_pool(name="pep", bufs=1))
        pz = ctx.enter_context(tc.tile_pool(name="pz", bufs=1, space="PSUM"))
        ps = ctx.enter_context(tc.tile_pool(name="ps", bufs=1, space="PSUM"))

        # consts go through the SWDGE ring so the sync HWDGE ring starts the
        # big data streams immediately
        wt_s = pconst.tile([128, N_ETILES * NUM_CLASS], dt_data)
        nc.gpsimd.dma_start(out=wt_s, in_=wt)
        ones_s = pconst.tile([128, 1], dt_sq)
        nc.gpsimd.dma_start(out=ones_s, in_=onesv)
        sones_s = pconst.tile([1, NUM_CLASS], f32r)
        nc.gpsimd.dma_start(out=sones_s, in_=sones)
        sbias_s = pconst.tile([NUM_CLASS, 1], f32)
        nc.gpsimd.dma_start(out=sbias_s, in_=sbias)
        zero1_s = pconst.tile([1, 1], f32)
        nc.vector.memset(zero1_s, 0.0)
        zero128_s = pconst.tile([128, 1], f32)
        nc.vector.memset(zero128_s, 0.0)
        # rsqrt via exp(-0.5*ln(ss) + ln(S)): folds the *S scale in for free
        lnS_s = pconst.tile([1, 1], f32)
        nc.vector.memset(lnS_s, float(np.log(S)))

        # preload the one ACT table set that covers Square+Ln+Exp so no
        # table switch lands on the critical path (greedy per-function
        # selection would otherwise load 3 sets)
        nlx_id = list(get_activation_tables(nc.m.arch)).index(
            "natural_log_exp_and_others"
        )
        nc.scalar.add_instruction(
            mybir.InstLoadActFuncSet(
                name=f"I-{nc.next_id()}", act_func_set_id=nlx_id
            )
        )

        # accumulators: z.T as one 4-bank tensor (PE-only writers), sumsq as
        # four single-bank tensors so the epilogue rnb reuse pipelines
        zt_ps = pz.tile([NUM_CLASS, ROWS], f32, tag="zt")
        ss_ps = [
            ps.tile([1, 512], f32, tag="ssrnb", bufs=4, name=f"ss{j}")
            for j in range(N_BCHUNK)
        ]

        # panel groups: single-panel first DMAs so the PE starts early, then
        # 2-panel (2MB fp32) transfers for bandwidth
        groups = [(0,), (1,)] + [
            (t, t + 1) for t in range(2, N_ETILES, 2)
        ]
        for gi, g in enumerate(groups):
            npan = len(g)
            xt = pdata.tile([128, 2, ROWS], dt_data, tag="xt")
            # alternate the two HWDGE rings (SP and ACT) so transfers overlap
            dma_eng = nc.scalar if gi % 2 == 0 else nc.sync
            dma_eng.dma_start(
                out=xt[:, :npan, :],
                in_=feaT[g[0] * 128 : (g[-1] + 1) * 128, :].rearrange(
                    "(a p) b -> p a b", p=128
                ),
            )
            x2 = psq.tile([128, 2, ROWS], dt_sq, tag="x2")
            for a in range(npan):
                # alternate square engine per panel: ACT and DVE run them
                # concurrently
                xin = xt[:, a, :] if use_bf16 else xt[:, a, :].bitcast(f32)
                if g[a] % 2 == 0:
                    nc.scalar.activation(
                        out=x2[:, a, :],
                        in_=xin,
                        func=mybir.ActivationFunctionType.Square,
                        bias=zero128_s,
                        scale=1.0,
                    )
                else:
                    nc.vector.tensor_mul(x2[:, a, :], xin, xin)

            # z matmuls first (need only xt), ss matmuls after (need x2)
            for a in range(npan):
                t = g[a]
                start = t == 0
                stop = t == N_ETILES - 1
                wchunk = wt_s[:, t * NUM_CLASS : (t + 1) * NUM_CLASS]
                for j in range(N_BCHUNK):
                    bsl = slice(j * 512, (j + 1) * 512)
                    nc.tensor.matmul(
                        zt_ps[:, bsl], wchunk, xt[:, a, bsl], start=start, stop=stop
                    )
            for a in range(npan):
                t = g[a]
                start = t == 0
                stop = t == N_ETILES - 1
                for j in range(N_BCHUNK):
                    bsl = slice(j * 512, (j + 1) * 512)
                    nc.tensor.matmul(
                        ss_ps[j], ones_s, x2[:, a, bsl], start=start, stop=stop
                    )

        # epilogue: out.T[c,b] = z.T[c,b] * S/sqrt(sumsq[b]) + S*bias[c]
        # S/sqrt(ss) = exp(-0.5*ln(ss) + ln(S)): two 1-cyc/elem ACT ops from
        # one table set, avoiding the slow iterative DVE reciprocal.  The
        # multiply and bias-add run on DVE so ACT only ever loads one set.
        lnss_s = pep.tile([1, ROWS], f32)
        rnorm_s = pep.tile([1, ROWS], f32r)
        z_s = pep.tile([NUM_CLASS, ROWS], f32)
        zr_s = pep.tile([NUM_CLASS, ROWS], f32)
        out_s = pep.tile([NUM_CLASS, ROWS], f32)
        # copy z.T out of psum as soon as its accumulation stops
        nc.vector.tensor_copy(z_s, zt_ps)
        # per-chunk dependency chains, interleaved so chunk j reaches its
        # broadcast matmul as early as possible
        rnb = [
            ps.tile([NUM_CLASS, 512], f32, tag="ssrnb", bufs=4, name=f"rnb{j}")
            for j in range(N_BCHUNK)
        ]
        for j in range(N_BCHUNK):
            bsl = slice(j * 512, (j + 1) * 512)
            nc.scalar.activation(
                out=lnss_s[:, bsl],
                in_=ss_ps[j],
                func=mybir.ActivationFunctionType.Ln,
                bias=zero1_s,
                scale=1.0,
            )
            nc.scalar.activation(
                out=rnorm_s[:, bsl],
                in_=lnss_s[:, bsl],
                func=mybir.ActivationFunctionType.Exp,
                bias=lnS_s,
                scale=-0.5,
            )
            # broadcast S/norm across the 4 class partitions via a k=1
            # f32r matmul (single-pass PE; reuses a freed sumsq psum bank)
            nc.tensor.matmul(
                rnb[j], sones_s, rnorm_s[:, bsl], start=True, stop=True
            )
            nc.vector.tensor_mul(zr_s[:, bsl], z_s[:, bsl], rnb[j])
            nc.vector.tensor_scalar_add(
                out_s[:, bsl], in0=zr_s[:, bsl], scalar1=sbias_s
            )
            nc.sync.dma_start(
                out=outT[:, bsl], in_=out_s[:, bsl]
            )

    nc.compile()
    return nc


def _get_nc():
    if "nc" not in _CACHE:
        _CACHE["nc"] = _build_nc()
    return _CACHE["nc"]


def _stage_inputs(fea, W, b):
    import ml_dtypes

    np_data = ml_dtypes.bfloat16 if DTYPE_CFG == "bf16" else np.float32
    fea = np.asarray(fea, dtype=np.float32)
    W = np.asarray(W, dtype=np.float32)
    b = np.asarray(b, dtype=np.float32)

    # wt[p, 4t+c] = W[c, 128t+p]
    wt = np.ascontiguousarray(
        W.reshape(NUM_CLASS, N_ETILES, 128).transpose(2, 1, 0).reshape(128, -1)
    ).astype(np_data)
    onesv = np.ones((128, 1), dtype=ml_dtypes.bfloat16)
    # the *S scale is folded into the exp(-0.5*ln(ss)+ln(S)) rsqrt, so the
    # class-broadcast matmul uses plain ones
    sones = np.ones((1, NUM_CLASS), dtype=np.float32)
    sbias = (S * b).reshape(NUM_CLASS, 1).astype(np.float32)

    in_maps = []
    for i in range(N_CORES):
        shard = fea[i * ROWS : (i + 1) * ROWS, :]
        feaT = np.ascontiguousarray(shard.T).astype(np_data)
        in_maps.append(
            {"feaT": feaT, "wt": wt, "onesv": onesv, "sones": sones, "sbias": sbias}
        )
    return in_maps


def run(fea, W, b, trace=False):
    from concourse.bass_utils import run_bass_kernel_spmd

    nc = _get_nc()
    in_maps = _stage_inputs(fea, W, b)
    res = run_bass_kernel_spmd(
        nc, in_maps, core_ids=list(range(N_CORES)), trace=trace
    )
    out = np.empty((BATCH, NUM_CLASS), dtype=np.float32)
    for i in range(N_CORES):
        out[i * ROWS : (i + 1) * ROWS, :] = res.results[i]["outT"].T
    return out, res


def kernel(fea, W, b):
    out, _ = run(fea, W, b, trace=False)
    return out
